# revision 19
# baseline (speedup 1.0000x reference)
"""MiniRocket feature extractor on 8 Trainium2 NeuronCores (optimized).

Per core (4 batch rows), per (dilation, batch) unit:
  - one strided DMA builds xshift [72, 4096] fp16 in SBUF (9 dilated tap
    shifts x 8 channels of the zero-padded series),
  - TensorE computes resp = W^T @ xshift (contraction 72 = channel x tap),
  - ScalarE drains PSUM fp32 -> SBUF int16 with a per-kernel scale chosen
    so all four bias thresholds sit inside +/-32000.  Units are packed 3
    to 2 dense [<=128, 4096] tiles; the row0=0 segment of each even tile
    is drained DIRECTLY into the packed tile, the misaligned segments go
    through per-unit tiles and an SBUF->SBUF SWDGE repack,
  - VectorE counts the pad-edge strips per threshold (strided 3D pass over
    both edges at once) from the unit's rows,
  - VectorE runs fused compare+count (is_gt + add-reduce) per threshold on
    the packed tiles with per-partition threshold LUT columns (full 128-
    partition utilization vs 84/96 unpacked),
  - raw count accumulators are DMA'd out once at the end; the trim-parity /
    normalization finalize is linear in the counts and runs on the host.
"""

import numpy as np
from contextlib import ExitStack

import concourse.bass as bass
import concourse.mybir as mybir
import concourse.tile as tile
from concourse.ap import AP
from concourse.bass_utils import run_bass_kernel_spmd

DILATIONS = (1, 2, 4, 8, 16, 32)
ND = 6
K = 84
KS = 9
C = 8
L = 4096
F = 4
B = 32
N_CORES = 8
B_LOC = 4
PADMAX = 128
LP = L + 2 * PADMAX
KP = 96
NFEAT = ND * K * F
NU = ND * B_LOC          # 24 units per core
NG = NU // 3             # 8 groups of 3 units -> 2 tiles each
NT = 2 * NG              # 16 packed tiles
HC = L // 2              # 2048 cols per drain half

F32 = mybir.dt.float32
F16 = mybir.dt.float16
I16 = mybir.dt.int16


def _pack_map():
    """tiles[t] = (nrows, [(unit, k0, k1, row0), ...]); row0==0 & k==0:84
    segments are drained directly into the packed tile."""
    tiles = []
    for g in range(NG):
        u0, u1, u2 = 3 * g, 3 * g + 1, 3 * g + 2
        # the group's LAST unit takes the direct (row0=0, no-repack) slot:
        # its tile is compare-ready right at drain-end (no repack latency)
        tiles.append((128, [(u2, 0, 84, 0), (u0, 0, 44, 84)]))
        tiles.append((124, [(u0, 44, 84, 0), (u1, 0, 84, 40)]))
    return tiles


PACK = _pack_map()


def _is_direct(seg):
    u, k0, k1, row0 = seg
    return row0 == 0 and k0 == 0 and k1 == 84


def _split_excess_waits(nc, max_waits=1):
    """This walrus build allows only one sync-wait per instruction; hoist
    extra waits onto preceding NOPs of the same engine."""
    n = 0
    for f in nc.m.functions:
        for bb in f.blocks:
            insts = bb.instructions
            if not any(
                i.sync_info and i.sync_info.on_wait and len(i.sync_info.on_wait) > max_waits
                for i in insts
            ):
                continue
            out = []
            for inst in insts:
                si = inst.sync_info
                waits = list(si.on_wait) if si and si.on_wait else []
                if len(waits) > max_waits:
                    for w in waits[:-max_waits]:
                        nop = mybir.InstNoOp(name=f"syncfix-{n}", ins=[], outs=[])
                        n += 1
                        nop.engine = inst.engine
                        nop.sync_info = mybir.SyncInfo(on_wait=[w], on_update=[])
                        out.append(nop)
                    inst.sync_info = mybir.SyncInfo(
                        on_wait=waits[-max_waits:],
                        on_update=list(si.on_update or []),
                    )
                out.append(inst)
            bb.instructions = out


def _build_nc():
    nc = bass.Bass()
    xprep = nc.declare_dram_parameter("xprep", [B_LOC, C, LP], F16, isOutput=False)
    wstack = nc.declare_dram_parameter("wstack", [ND, 72, KP], F16, isOutput=False)
    # cpack: [KP, 30] = 24 cols edge-bias (d-major, f-minor) + 6 cols drain scale
    cpack = nc.declare_dram_parameter("cpack", [KP, 30], F32, isOutput=False)
    # blut: per packed tile per-partition thresholds, col t*4+f
    blut = nc.declare_dram_parameter("blut", [128, NT * F], F32, isOutput=False)
    acc_out = nc.declare_dram_parameter("acc_out", [128, NT * F], F32, isOutput=True)
    eacc_out = nc.declare_dram_parameter("eacc_out", [KP, NU * F], F32, isOutput=True)

    alu = mybir.AluOpType

    # unit -> (tile, seg) it contributes to
    unit_segs = {}
    tile_done_at = {}
    for t, (nrows, segs) in enumerate(PACK):
        for seg in segs:
            unit_segs.setdefault(seg[0], []).append((t, seg))
        tile_done_at[t] = max(s[0] for s in segs)

    with tile.TileContext(nc) as tc, ExitStack() as ctx:
        cpool = ctx.enter_context(tc.tile_pool(name="const", bufs=1))
        xsh_pool = ctx.enter_context(tc.tile_pool(name="xsh", bufs=7))
        psum_pool = ctx.enter_context(tc.tile_pool(name="psum", bufs=2, space="PSUM"))
        resp_pool = ctx.enter_context(tc.tile_pool(name="resp", bufs=5))
        pk_pool = ctx.enter_context(tc.tile_pool(name="pk", bufs=5))
        trash_pool = ctx.enter_context(tc.tile_pool(name="trash", bufs=1))

        # small consts first on the queue (fast; unblocks first drains early)
        w_t = cpool.tile([72, ND * KP], F16)
        nc.sync.dma_start(w_t[:], AP(wstack, 0, [[KP, 72], [72 * KP, ND], [1, KP]]))
        cp_t = cpool.tile([KP, 30], F32, tag="cp_t")
        nc.sync.dma_start(cp_t[:], AP(cpack, 0, [[30, KP], [1, 30]]))
        ebias_t = cp_t[:, 0:24]
        ss_t = cp_t[:, 24:30]

        # first two input loads next
        early_xsh = {}
        for b0 in range(2):
            xsh = xsh_pool.tile([72, L], F16)
            nc.sync.dma_start(
                xsh[:],
                AP(xprep, b0 * C * LP + (PADMAX - 4), [[1, KS], [LP, C], [1, L]]),
            )
            early_xsh[b0] = xsh

        bl_t = cpool.tile([128, NT * F], F32, tag="bl_t")
        nc.sync.dma_start(bl_t[:], AP(blut, 0, [[NT * F, 128], [1, NT * F]]))

        acc_t = cpool.tile([128, NT * F], F32, tag="acc_t")
        eacc_t = cpool.tile([KP, NU * F], F32, tag="eacc_t")

        trash_es = [
            trash_pool.tile([KP, 256], I16, tag=f"trash_e{i}", name=f"trash_e{i}")
            for i in range(4)
        ]
        trash_ps = [
            trash_pool.tile([128, L], I16, tag=f"trash_p{i}", name=f"trash_p{i}")
            for i in range(4)
        ]

        pk_tiles = {}      # tile idx -> packed tile

        def emit_cmps(t):
            nrows, segs = PACK[t]
            pk = pk_tiles[t]
            for f in range(F):
                nc.vector.tensor_scalar(
                    trash_ps[f][0:nrows, :], pk[0:nrows, :],
                    bl_t[0:nrows, t * F + f : t * F + f + 1], None,
                    alu.is_gt, alu.add,
                    accum_out=acc_t[0:nrows, t * F + f : t * F + f + 1],
                )
            del pk_tiles[t]

        for di, d in enumerate(DILATIONS):
            pad = 4 * d
            w_d = w_t[:, di * KP : (di + 1) * KP]
            for b in range(B_LOC):
                u = di * B_LOC + b
                if di == 0 and b in early_xsh:
                    xsh = early_xsh[b]
                else:
                    xsh = xsh_pool.tile([72, L], F16)
                    nc.sync.dma_start(
                        xsh[:],
                        AP(xprep, b * C * LP + (PADMAX - pad), [[d, KS], [LP, C], [1, L]]),
                    )

                # direct units (one 84-row aligned segment) drain straight
                # into the packed tile; others go via a per-unit tile + DMA
                (t_of_u, seg) = unit_segs[u][0]
                direct = len(unit_segs[u]) == 1 and _is_direct(seg)
                if direct:
                    if t_of_u not in pk_tiles:
                        pk_tiles[t_of_u] = pk_pool.tile([128, L], I16, name="pk")
                    dest = pk_tiles[t_of_u]
                    nrows_d = 84
                else:
                    dest = resp_pool.tile([KP, L], I16, name="resp16")
                    nrows_d = KP

                for h in range(2):
                    ps = psum_pool.tile([KP, HC], F32)
                    for n in range(4):
                        nc.tensor.matmul(
                            ps[:, n * 512 : (n + 1) * 512],
                            w_d,
                            xsh[:, h * HC + n * 512 : h * HC + (n + 1) * 512],
                            start=True,
                            stop=True,
                        )
                    if u < 3 and h == 1:
                        # h1 drains of the first three units go on VectorE:
                        # fills DVE's startup idle, shortens the ScalarE
                        # stream, and keeps Act starting early on h0
                        nc.vector.tensor_scalar_mul(
                            dest[0:nrows_d, h * HC : (h + 1) * HC],
                            ps[0:nrows_d, :],
                            ss_t[0:nrows_d, di : di + 1],
                        )
                    else:
                        nc.scalar.activation(
                            dest[0:nrows_d, h * HC : (h + 1) * HC],
                            ps[0:nrows_d, :],
                            mybir.ActivationFunctionType.Copy,
                            scale=ss_t[0:nrows_d, di : di + 1],
                        )

                # compares for tiles completed by the PREVIOUS unit go FIRST
                # on the DVE queue: they are ready before this unit's drains
                # finish, so they fill the drain window (no head-of-line wait)
                for t in range(NT):
                    if tile_done_at[t] == u - 2 and t in pk_tiles:
                        emit_cmps(t)

                # edge counts from this unit's rows (both strips, one pass)
                col0 = di * F
                pstep = dest[:].ap[0][0]
                for f in range(F):
                    b_ap = ebias_t[0:nrows_d, col0 + f : col0 + f + 1]
                    ein = AP(
                        dest[:].tensor, dest[:].offset,
                        [[pstep, nrows_d], [L - pad, 2], [1, pad]],
                    )
                    te = trash_es[f]
                    eout = AP(
                        te[:].tensor, te[:].offset,
                        [[te[:].ap[0][0], nrows_d], [pad, 2], [1, pad]],
                    )
                    nc.vector.tensor_scalar(
                        eout, ein, b_ap, None, alu.is_gt, alu.add,
                        accum_out=eacc_t[0:nrows_d, u * F + f : u * F + f + 1],
                    )

                # repack misaligned segments into their packed tiles
                if not direct:
                    for (t, (uu, k0, k1, row0)) in unit_segs[u]:
                        if t not in pk_tiles:
                            pk_tiles[t] = pk_pool.tile([128, L], I16, name="pk")
                        pk = pk_tiles[t]
                        for h in range(2):
                            nc.gpsimd.dma_start(
                                pk[row0 : row0 + (k1 - k0), h * HC : (h + 1) * HC],
                                dest[k0:k1, h * HC : (h + 1) * HC],
                            )


        for t in sorted(pk_tiles.keys(), key=lambda t: tile_done_at[t]):
            emit_cmps(t)

        # accumulator writebacks at the end
        nc.sync.dma_start(AP(acc_out, 0, [[NT * F, 128], [1, NT * F]]), acc_t[:])
        nc.sync.dma_start(AP(eacc_out, 0, [[NU * F, KP], [1, NU * F]]), eacc_t[:])

    _split_excess_waits(nc)
    return nc


_NC_CACHE = None


def _get_nc():
    global _NC_CACHE
    if _NC_CACHE is None:
        _NC_CACHE = _build_nc()
    return _NC_CACHE


LAST_RESULTS = None


def kernel(x, channel_masks, bias_matrices, feature_mean, feature_std):
    global LAST_RESULTS
    x = np.ascontiguousarray(np.asarray(x, dtype=np.float32))
    masks = np.asarray(channel_masks, dtype=np.float32)
    biasm = np.asarray(bias_matrices, dtype=np.float32)
    mean = np.asarray(feature_mean, dtype=np.float32)
    std = np.asarray(feature_std, dtype=np.float32)

    wstack = np.zeros((ND, 72, KP), np.float16)
    for di in range(ND):
        wt = -masks[di].T.astype(np.float16)
        for j in range(KS):
            wstack[di, j * C : (j + 1) * C, :K] = wt
    # int16 drain: resp is stored as round(resp * s_dk); compare against
    # bias * s_dk. s_dk chosen so all four thresholds sit inside +/-32000.
    sscale = np.zeros((ND, KP), np.float32)
    maxb = np.maximum(np.abs(biasm).max(axis=-1), 1e-6)  # [ND, K]
    sscale[:, :K] = 32000.0 / maxb
    biases_pad = np.full((ND, KP, F), 32100.0, np.float32)
    biases_pad[:, :K, :] = biasm * sscale[:, :K, None]

    xt = np.ascontiguousarray(x.transpose(0, 2, 1))
    xp = np.zeros((B, C, LP), np.float16)
    xp[:, :, PADMAX : PADMAX + L] = xt.astype(np.float16)

    cpk = np.zeros((KP, 30), np.float32)
    cpk[:, 0:24] = biases_pad.transpose(1, 0, 2).reshape(KP, 24)
    cpk[:, 24:30] = sscale.T

    # packed-tile threshold LUT [128, NT*F]
    blut_np = np.full((128, NT * F), 32100.0, np.float32)
    for t, (nrows, segs) in enumerate(PACK):
        for (u, k0, k1, row0) in segs:
            di = u // B_LOC
            for f in range(F):
                blut_np[row0 : row0 + (k1 - k0), t * F + f] = biases_pad[di, k0:k1, f]

    nc = _get_nc()
    in_maps = []
    for core in range(N_CORES):
        in_maps.append(
            {
                "xprep": np.ascontiguousarray(xp[core * B_LOC : (core + 1) * B_LOC]),
                "wstack": wstack,
                "cpack": cpk,
                "blut": blut_np,
            }
        )
    res = run_bass_kernel_spmd(nc, in_maps, list(range(N_CORES)))
    LAST_RESULTS = res

    # host finalize: decode packed accumulators, then
    # counts -> (ppv - mean) / std with trim parity
    full_cnt = np.zeros((B, ND, K, F), np.float64)
    edge_cnt = np.zeros((B, ND, K, F), np.float64)
    for core in range(N_CORES):
        acc = np.asarray(res.results[core]["acc_out"], np.float64)    # [128, NT*F]
        eacc = np.asarray(res.results[core]["eacc_out"], np.float64)  # [KP, NU*F]
        for t, (nrows, segs) in enumerate(PACK):
            for (u, k0, k1, row0) in segs:
                di, b = u // B_LOC, u % B_LOC
                rows = slice(row0, row0 + (k1 - k0))
                full_cnt[core * B_LOC + b, di, k0:k1, :] += acc[rows, t * F : (t + 1) * F]
        for u in range(NU):
            di, b = u // B_LOC, u % B_LOC
            edge_cnt[core * B_LOC + b, di] = eacc[:K, u * F : (u + 1) * F]

    feats = np.zeros((B, ND, K, F), np.float64)
    kk = np.arange(K)
    for di, d in enumerate(DILATIONS):
        pad = 4 * d
        lt = L - 2 * pad
        parity = ((di + kk) % 2 == 1)[None, :, None]  # [1, K, 1]
        mean_full = full_cnt[:, di] / L
        mean_trim = (full_cnt[:, di] - edge_cnt[:, di]) / lt
        feats[:, di] = np.where(parity, mean_trim, mean_full)
    flat = feats.reshape(B, NFEAT).astype(np.float32)
    return ((flat - mean[None, :]) / std[None, :]).astype(np.float32)
